# revision 30
# baseline (speedup 1.0000x reference)
"""BiasAttention TRN2 kernel — q-sharded across 8 NeuronCores, fp8 z.

Each core owns 128 queries and computes full attention for them (8 heads,
1024 keys) with no collectives.  The dominant cost is streaming the bias
tensor z ([q,k,c] = 16.8 MB/core in fp8): host prep casts z to e3m4
(4 mantissa bits; z~N(0,1) fits the +-15.5 range) and lays it out
[kc, c, q, k] so each [c=128, k=128] slice is LDWEIGHTS-ready; fp8
weight loads stream at ~27 ns/tile (4 B/cycle fast path).  The z stream
runs as 1 MB transfers alternating between the two HWDGE rings
(sync/scalar) with a 6-deep tile pool, z chunk 0 issued before anything
else on its ring; all bf16 weights ride in a single packed transfer and
the fp32 oddments in another, so ring fixed costs are paid ~3x, not ~12x.

Scores are computed directly in the transposed frame S^T[k, q] (lhsT = K^T)
so the exp output feeds the AV matmul without any PE transposes.  Q^T/K^T
are stored per-head on partitions 0-31 ([33, h, n]; row 32 folds the bb
bias into the score matmul: kT row = 1, qT row = 64*bb[h]) so score
matmuls never need tile_position (multi-matmul PSUM accumulation with
row-positioned strips aborts at load on this runtime).  Per k-chunk: two
score banks [k, (4h, 128q)] and two bias banks [k, (64q, 8h)] of 64 z
matmuls each (rhs = e3m4(64*Wb)).  The 1/64 Wb scale (needed to lift Wb
out of e3m4's subnormal range) is folded into the exp activations via
exp(a+b) = exp(a)*exp(b): each PSUM bank gets its own exp (ACT reads one
PSUM operand) and a DVE multiply fuses them into bf16 slabs that are the
AV lhsT as-is; a single whole-tile rewrite then feeds the AV weight loads
(two half-writers race with the PE LDWEIGHTS pull-ahead on hardware).
AV for chunk kc runs during chunk kc+2 so the ACT/DVE chain never stalls
the PE, and the V projection + second half of the K projection are
deferred into the first chunks' bodies to fill the z-stream ramp.  A ones
column appended to V accumulates the softmax denominator in the AV pass.
"""

import sys

if "/opt/trn_rl_repo" not in sys.path:
    sys.path.insert(0, "/opt/trn_rl_repo")

import ml_dtypes
import numpy as np

import concourse.bass as bass
import concourse.mybir as mybir
from concourse import bacc
from concourse.bass_utils import run_bass_kernel_spmd
from concourse.masks import make_identity
from concourse.tile import TileContext

P = 128          # partitions
H = 8            # heads
D = 32           # head dim
CQ = 256         # q channels
CKV = 256        # kv channels
BD = 128         # bias (z) channels
NQ = 1024        # total queries
NK = 1024        # total keys
NCORES = 8
NQC = NQ // NCORES   # queries per core = 128
KC_N = NK // P       # k-chunks of 128
QH = NQC // 2        # queries per bias psum bank = 64
SCALE = D ** (-0.5)
WBS = 64.0           # Wb host-side scale (power of two)

# packed bf16 weights blob: [P, 2, WPK] = wq | xqT | wkv | xkvT
WPK = H * D + NQC + 2 * H * D + NK
WOFF_Q = 0
WOFF_XQ = H * D
WOFF_KV = H * D + NQC
WOFF_XKV = H * D + NQC + 2 * H * D
# packed fp32 blob: [P, FPK] = wp (2*CQ) | bq2 | bkvK2 | bp
FPK = 2 * CQ + 8 + 8 + CQ

FP = mybir.dt.float32
BF = mybir.dt.bfloat16
F8 = mybir.dt.float8e3
NP_BF = ml_dtypes.bfloat16
NP_F8 = ml_dtypes.float8_e3m4


def build_program():
    add = mybir.AluOpType.add
    mult = mybir.AluOpType.mult

    nc = bacc.Bacc("TRN2", target_bir_lowering=False, debug=False,
                   num_devices=NCORES)

    # ---- I/O ----
    zT = nc.dram_tensor("zT", [KC_N, BD, NQC, P], F8, kind="ExternalInput")
    wpack = nc.dram_tensor("wpack", [P, 2, WPK], BF, kind="ExternalInput")
    fpack = nc.dram_tensor("fpack", [P, FPK], FP, kind="ExternalInput")
    bpack = nc.dram_tensor("bpack", [1, H * D + H * QH], BF,
                           kind="ExternalInput")
    Wb64 = nc.dram_tensor("Wb64", [BD, H], F8, kind="ExternalInput")
    y = nc.dram_tensor("y", [NQC, CQ], FP, kind="ExternalOutput")

    with TileContext(nc) as tc:
        with (
            tc.tile_pool(name="const", bufs=1) as const,
            tc.tile_pool(name="zpool", bufs=6) as zpool,
            tc.tile_pool(name="epool", bufs=3) as epool,
            tc.tile_pool(name="proj_ps", bufs=2, space="PSUM") as proj_ps,
            tc.tile_pool(name="b_ps", bufs=3, space="PSUM") as b_psp,
            tc.tile_pool(name="s_ps", bufs=2, space="PSUM") as s_psp,
            tc.tile_pool(name="o_ps", bufs=1, space="PSUM") as o_psp,
        ):
            # ---- weights first on the scalar ring; the z stream runs as
            # 512 KB quarter-chunks alternating rings so both rings feed
            # every chunk and transfers stay deeply queued ----
            z_tiles = {}

            def z_dma(gidx):
                zn = zpool.tile([P, NQC, P], F8, tag="z", name=f"z{gidx}")
                for quart in range(4):
                    eng = nc.sync if (gidx * 4 + quart) % 2 == 0 else nc.scalar
                    eng.dma_start(zn[:, quart * 32:(quart + 1) * 32, :],
                                  zT[gidx, :, quart * 32:(quart + 1) * 32, :])
                z_tiles[gidx] = zn

            wpack_sb = const.tile([P, 2, WPK], BF)
            nc.scalar.dma_start(wpack_sb, wpack[:])
            fpack_sb = const.tile([P, FPK], FP)
            nc.scalar.dma_start(fpack_sb, fpack[:])
            bpack_sb = const.tile([1, H * D + H * QH], BF)
            nc.scalar.dma_start(bpack_sb, bpack[:])
            wb_sb = const.tile([P, H], F8)
            nc.scalar.dma_start(wb_sb, Wb64[:])
            z_dma(0)
            z_dma(1)
            z_dma(2)
            z_dma(3)

            wq_sb = wpack_sb[:, :, WOFF_Q:WOFF_Q + H * D]
            xqT_sb = wpack_sb[:, :, WOFF_XQ:WOFF_XQ + NQC]
            wkv_sb = wpack_sb[:, :, WOFF_KV:WOFF_KV + 2 * H * D]
            xkvT_sb = wpack_sb[:, :, WOFF_XKV:WOFF_XKV + NK]
            wp_sb = fpack_sb[:, 0:2 * CQ].rearrange("p (o m) -> p o m", o=2)
            bq2_sb = fpack_sb[0:D, 2 * CQ:2 * CQ + H]
            bkvK2_sb = fpack_sb[0:D, 2 * CQ + H:2 * CQ + 2 * H]
            bp_sb = fpack_sb[0:1, 2 * CQ + 2 * H:2 * CQ + 2 * H + CQ]
            bkvV_sb = bpack_sb[:, 0:H * D]
            bbpat_sb = bpack_sb[:, H * D:]

            ident = const.tile([P, P], FP)
            make_identity(nc, ident)
            ones_bf = const.tile([1, P], BF)
            nc.vector.memset(ones_bf, 1.0)

            # PE warmup while the first DMAs land: un-throttle the HAM clock
            # gate with identity matmuls so the real work runs at 2.4 GHz
            warm_ps = proj_ps.tile([P, 512], FP, tag="proj", name="warm_ps")
            for w in range(14):
                nc.tensor.matmul(warm_ps[:, :P], lhsT=ident, rhs=ident,
                                 start=(w == 0), stop=(w == 13))

            # V augmented with a ones column per head: [k, kc, h, D+1]
            vaug_sb = const.tile([P, KC_N, H, D + 1], BF)
            nc.vector.memset(vaug_sb, 1.0)

            # ---- projections (bf16 in, fp32 psum accumulate) ----
            # per-head Q^T/K^T on partitions 0-31: [32, h, n]
            qT_sb = const.tile([D, H, NQC], BF)
            kT_sb = const.tile([D, H, NK], BF)
            for h in range(H):
                ps = proj_ps.tile([P, 512], FP, tag="proj", name="q_ps")
                for c in range(2):
                    nc.tensor.matmul(ps[0:D, 0:NQC],
                                     lhsT=wq_sb[:, c, h * D:(h + 1) * D],
                                     rhs=xqT_sb[:, c, :],
                                     start=(c == 0), stop=(c == 1))
                nc.vector.tensor_scalar(qT_sb[0:D, h, :], ps[0:D, 0:NQC],
                                        bq2_sb[:, h:h + 1], WBS * SCALE,
                                        add, mult)

            def emit_kproj(k0, kn):
                for h in range(H):
                    ps = proj_ps.tile([P, 512], FP, tag="proj", name="k_ps")
                    for c in range(2):
                        nc.tensor.matmul(ps[0:D, 0:kn - k0],
                                         lhsT=wkv_sb[:, c, h * D:(h + 1) * D],
                                         rhs=xkvT_sb[:, c, k0:kn],
                                         start=(c == 0), stop=(c == 1))
                    nc.vector.tensor_copy(
                        kT_sb[0:D, h, k0:kn], ps[0:D, 0:kn - k0])

            def emit_vproj(vkc):
                # V [k, (h d)] + bkv_V, written into vaug (ones col kept)
                ps = proj_ps.tile([P, 512], FP, tag="proj", name="v_ps")
                for c in range(2):
                    nc.tensor.matmul(ps[:, :H * D],
                                     lhsT=xkvT_sb[:, c, vkc * P:(vkc + 1) * P],
                                     rhs=wkv_sb[:, c, H * D:2 * H * D],
                                     start=(c == 0), stop=False)
                nc.tensor.matmul(ps[:, :H * D], lhsT=ones_bf,
                                 rhs=bkvV_sb, start=False, stop=True)
                nc.scalar.activation(
                    vaug_sb[:, vkc, :, 0:D],
                    ps[:, :H * D].rearrange("p (h d) -> p h d", h=H),
                    mybir.ActivationFunctionType.Copy)

            # only kc0's 128 keys gate chunk 0; the rest of K^T and all
            # of V fill the PE while the z stream ramps up
            emit_kproj(0, P)

            # ---- main loop over k-chunks ----
            o_ps = o_psp.tile([P, H * (D + 1)], FP)
            e_hist = []    # [(kc, [e2_hg0, e2_hg1])], AV lags 2 chunks

            def emit_av(pkc, pe, first, last):
                for h in range(H):
                    nc.tensor.matmul(
                        o_ps[:, h * (D + 1):(h + 1) * (D + 1)],
                        lhsT=pe[h // 4][:, h % 4, :],
                        rhs=vaug_sb[:, pkc, h, :],
                        start=(first and h == 0),
                        stop=(last and h == H - 1))

            for kc in range(KC_N):
                nxt = kc + 4
                if nxt < KC_N:
                    z_dma(nxt)
                z_sb = z_tiles.pop(kc)

                # S^T banks [k, (4h, 128q)], strip-0 matmuls only
                s_tiles = []
                for hg in range(2):
                    s_ps = s_psp.tile([P, 4, NQC], FP, tag="s")
                    for hl in range(4):
                        h = hg * 4 + hl
                        nc.tensor.matmul(
                            s_ps[:, hl, :],
                            lhsT=kT_sb[:, h, kc * P:(kc + 1) * P],
                            rhs=qT_sb[:, h, :],
                            start=(hl == 0), stop=(hl == 3))
                    s_tiles.append(s_ps)

                # bias banks [k, (64q, 8h)]: 64 z matmuls each; AV for
                # chunk kc-2 and deferred projections fill the gaps
                b_tiles = []
                for hf in range(2):
                    # bb[h] and q.bkvK are constant across k, so the
                    # softmax cancels them - no bias row needed at all
                    b_ps = b_psp.tile([P, QH, H], FP, tag="b")
                    for qi in range(QH):
                        nc.tensor.matmul(b_ps[:, qi, :],
                                         lhsT=z_sb[:, hf * QH + qi, :],
                                         rhs=wb_sb,
                                         start=(qi == 0), stop=(qi == QH - 1))
                    b_tiles.append(b_ps)
                    if hf == 0 and kc == 0:
                        emit_kproj(P, 512)
                        emit_kproj(512, NK)
                        for vkc in range(4):
                            emit_vproj(vkc)
                    if hf == 0 and kc == 1:
                        for vkc in range(4, KC_N):
                            emit_vproj(vkc)
                    if hf == 0 and len(e_hist) >= 2:
                        pkc, pe = e_hist.pop(0)
                        emit_av(pkc, pe, first=(pkc == 0), last=False)

                # exp((S'+bias'+bb')/64) = exp(S'/64) * exp((bias'+bb')/64)
                es, eb, e_sb = [], [], []
                for half in range(2):
                    t = epool.tile([P, 4, NQC], FP, tag=f"es{half}",
                                   name=f"es{half}")
                    nc.scalar.activation(t, s_tiles[half],
                                         mybir.ActivationFunctionType.Exp,
                                         scale=1.0 / WBS)
                    es.append(t)
                    t2 = epool.tile([P, QH, H], FP, tag=f"eb{half}",
                                    name=f"eb{half}")
                    nc.scalar.activation(t2, b_tiles[half],
                                         mybir.ActivationFunctionType.Exp,
                                         scale=1.0 / WBS)
                    eb.append(t2)
                for hg in range(2):
                    t = epool.tile([P, 4, NQC], BF, tag=f"e{hg}",
                                   name=f"e{hg}")
                    e_sb.append(t)
                for hg in range(2):
                    for hf in range(2):
                        nc.vector.tensor_tensor(
                            e_sb[hg][:, :, hf * QH:(hf + 1) * QH],
                            es[hg][:, :, hf * QH:(hf + 1) * QH],
                            eb[hf][:, :, hg * 4:(hg + 1) * 4]
                               .rearrange("p q h -> p h q"),
                            mult)
                # single whole-tile rewrite: the AV weight loads must wait on
                # ONE writer covering the full tile (two half-writers race
                # with the PE LDWEIGHTS pull-ahead on hardware)
                e2_sb = []
                for hg in range(2):
                    t = epool.tile([P, 4, NQC], BF, tag=f"e2{hg}",
                                   name=f"e2{hg}")
                    nc.vector.tensor_copy(t, e_sb[hg])
                    e2_sb.append(t)
                e_hist.append((kc, e2_sb))

            # drain the AV pipeline
            while e_hist:
                pkc, pe = e_hist.pop(0)
                emit_av(pkc, pe, first=(pkc == 0), last=(pkc == KC_N - 1))

            # ---- epilogue: normalize, transpose, output projection ----
            recip_sb = const.tile([P, H], FP)
            nc.vector.reciprocal(
                recip_sb, o_ps.rearrange("p (h x) -> p h x", x=D + 1)[:, :, D])
            recipb_ap = bass.AP(tensor=recip_sb.tensor, offset=recip_sb.offset,
                                ap=list(recip_sb.ap[:1]) + [[1, H], [0, D]])
            o_sb = const.tile([P, 2, P], FP)     # [q, half, (h d)%128]
            nc.vector.tensor_tensor(
                o_sb.rearrange("p o m -> p (o m)")
                    .rearrange("p (h d) -> p h d", h=H),
                o_ps.rearrange("p (h x) -> p h x", x=D + 1)[:, :, 0:D],
                recipb_ap, mult)
            oT_sb = const.tile([P, 2, P], FP)
            for m in range(2):
                t_full = proj_ps.tile([P, 512], FP, tag="proj", name="t_full")
                t_ps = t_full[:, :P]
                nc.tensor.transpose(t_ps, o_sb[:, m, :], ident)
                nc.vector.tensor_copy(oT_sb[:, m, :], t_ps)
            ones_fp = const.tile([1, P], FP)
            nc.vector.memset(ones_fp, 1.0)
            y_sb = const.tile([P, CQ], FP)
            for cq in range(2):
                ps = proj_ps.tile([P, 512], FP, tag="proj", name=f"y_ps{cq}")
                for m in range(2):
                    nc.tensor.matmul(ps[:, :P], lhsT=oT_sb[:, m, :],
                                     rhs=wp_sb[:, m, cq * P:(cq + 1) * P],
                                     start=(m == 0), stop=False)
                nc.tensor.matmul(ps[:, :P], lhsT=ones_fp,
                                 rhs=bp_sb[:, cq * P:(cq + 1) * P],
                                 start=False, stop=True)
                nc.vector.tensor_copy(y_sb[:, cq * P:(cq + 1) * P], ps[:, :P])
                eng = nc.sync if cq == 0 else nc.scalar
                eng.dma_start(y[:, cq * P:(cq + 1) * P],
                              y_sb[:, cq * P:(cq + 1) * P])

    nc.compile()
    return nc


def prep_inputs(x_q, x_kv, z, Wq, bq, Wkv, bkv, Wb, bb, Wp, bp):
    """Host-side shard prep.  Returns in_maps for the 8 cores."""
    def split2(a):      # [(2*P), n] -> [P, 2, n]
        return np.ascontiguousarray(
            np.asarray(a).reshape(2, P, -1).transpose(1, 0, 2))

    xkvT = np.asarray(x_kv[0]).T                             # [CKV, nk]
    wpack_np = np.empty((P, 2, WPK), dtype=NP_BF)
    wpack_np[:, :, WOFF_Q:WOFF_Q + H * D] = split2(Wq)
    wpack_np[:, :, WOFF_KV:WOFF_KV + 2 * H * D] = split2(Wkv)
    wpack_np[:, :, WOFF_XKV:WOFF_XKV + NK] = split2(xkvT)

    fpack_np = np.zeros((P, FPK), dtype=np.float32)
    fpack_np[:, 0:2 * CQ] = np.asarray(Wp).reshape(2, P, CQ)\
        .transpose(1, 0, 2).reshape(P, 2 * CQ)
    fpack_np[0:D, 2 * CQ:2 * CQ + H] = np.asarray(bq).reshape(H, D).T
    fpack_np[0:D, 2 * CQ + H:2 * CQ + 2 * H] = \
        np.asarray(bkv)[:H * D].reshape(H, D).T
    fpack_np[0:1, 2 * CQ + 2 * H:] = np.asarray(bp)[None, :]

    bpack_np = np.empty((1, H * D + H * QH), dtype=NP_BF)
    bpack_np[0, 0:H * D] = np.asarray(bkv)[H * D:].astype(NP_BF)
    bpack_np[0, H * D:] = np.tile(np.asarray(bb) * WBS, QH).astype(NP_BF)

    shared = dict(fpack=fpack_np, bpack=bpack_np,
                  Wb64=np.ascontiguousarray(Wb * WBS).astype(NP_F8))
    in_maps = []
    for i in range(NCORES):
        qs = i * NQC
        wp_i = wpack_np.copy()
        wp_i[:, :, WOFF_XQ:WOFF_XQ + NQC] = split2(
            np.asarray(x_q[0, qs:qs + NQC]).T)
        zi = z[0, qs:qs + NQC]                   # [q, k, c]
        # -> [kc, c, q, k128]
        zi = zi.reshape(NQC, KC_N, P, BD).transpose(1, 3, 0, 2)
        in_maps.append(dict(
            zT=np.ascontiguousarray(zi).astype(NP_F8),
            wpack=wp_i,
            **shared,
        ))
    return in_maps


_NC_CACHE = {}


def kernel(x_q, x_kv, z, Wq, bq, Wkv, bkv, Wb, bb, Wp, bp):
    key = "full"
    if key not in _NC_CACHE:
        _NC_CACHE[key] = build_program()
    nc = _NC_CACHE[key]
    in_maps = prep_inputs(x_q, x_kv, z, Wq, bq, Wkv, bkv, Wb, bb, Wp, bp)
    res = run_bass_kernel_spmd(nc, in_maps, list(range(NCORES)))
    out = np.empty((1, NQ, CQ), dtype=np.float32)
    for i in range(NCORES):
        out[0, i * NQC:(i + 1) * NQC, :] = res.results[i]["y"]
    return out
